# revision 48
# baseline (speedup 1.0000x reference)
"""Trainium2 Bass kernel for nn_GaussianLayer: ReflectionPad2d(10) +
depthwise 21x21 Gaussian conv on x:(16,3,512,512) f32.

Strategy
--------
The 21x21 Gaussian kernel is separable (rank-1): W[i,j] = wr[i]*wc[j].
Each (batch, channel) image is blurred with two 1D passes. Reflection
padding is folded into a precomputed 512x512 banded matrix (band width
21, edge taps folded by the reflection), so that per image

    y = B.T @ x @ B       (x, y: 512x512; B shared, Gaussian symmetric)

On the PE (out = lhsT.T @ rhs, contraction over the partition dim) both
passes keep the *image* stationary, which absorbs the transposes:

    pass 1: t1 = x.T @ B   (lhsT = x chunk,  rhs = B chunk)
    pass 2: y  = t1.T @ B  (lhsT = t1 chunk, rhs = B chunk)

Everything on the wire and in the PE is fp16 (PSUM accumulates fp32;
tolerance is 2e-2, fp16 end-to-end error ~6e-4):
  - x is converted to fp16 host-side and pre-permuted so each image is
    one DMA with 4KB contiguous runs per partition.
  - the band matrix is packed to just its nonzero columns per 128-row
    chunk (572 of 2048 columns) and sent fp16.
  - y is written fp16 and upcast host-side.
This cuts HBM traffic 13MB -> 6.2MB per core and runs matmuls at
1 cycle/row instead of fp32's 4.

Schedule (engine loads per image, cost-model ns):
  - PE 1907: both passes; pass 1 is j-outer over 4 concurrent PSUM
    banks so PE work is available as soon as each x half lands.
  - PSUM tiles are 2-bank pairs [128,2,512] so each PSUM->SBUF drain is
    one instruction (per-instruction init overhead halved). PSUM exits
    are the pipeline's cadence bound: only ACT (0.833 ns/elem + 185
    init) and DVE (1.042 + 125) may read PSUM (GPSIMD is verifier-
    blocked, DMA-from-PSUM is API-blocked, matmul output must be fp32),
    so per image ACT carries t1A+yA = 2076 and DVE t1B+yB = 2384; the
    2384 cadence is the floor at bank granularity. Do NOT merge the
    pairs into flat 4-bank tiles to balance-split the copies (2214/
    2214): that couples the ACT- and DVE-halves into one serial loop
    (p1(i+1) and p2(i+1) then wait BOTH engines' drains, ~3.8us/image,
    measured 35454ns). The pairs' independent half-chains are what let
    the pipeline run at per-engine work rate.
  - Dummy "warm" matmuls (into image 0's own PSUM pairs, overwritten by
    the real start=True groups) keep the PE busy from ~1.5us so the
    3us p-state ramp completes before real work, and fill the early
    input-DMA waits so the ramp never resets.
  - All input DMAs issue up-front on SP (the fastest DMA issuer; no
    sem waits -> no head-of-line blocking); images 0-1 are split in
    half (smaller pieces lose: sub-728ns transfers cannot stream
    through the 650ns HWDGE issue spacing). Outputs drain as per-pair
    DMAs emitted right behind the y copies.
  - The last image's pass-2 borrows the p1 PSUM banks (free for good
    once t1(last) drains) instead of waiting for image 4's y copies to
    release the p2 banks, pulling the tail in by ~0.9us.
Sharding: pure data parallel, 2 batches (6 images) per core across 8
cores. The band DMA rides Pool's SWDGE queue (its sequencer clears
before SP's prologue barrier), which keeps the input stream gapless.
The last two images run pass-2 r-outer so their pair-A groups stop
at the halfway mark and the final y copies start ~450ns sooner;
images 1-2 run pass-1 m-outer with the DVE pair first so t1B(1) —
the head of DVE's work-packed block — starts ~500ns sooner.
Best cost-model timeline: 25218 ns (baseline fp32 m-outer design:
57834 ns).
"""

import numpy as np

import concourse.bass as bass
import concourse.mybir as mybir
import concourse.tile as tile
from concourse.bass_utils import run_bass_kernel_spmd

KSIZE = 21
PAD = 10
H = 512
NBATCH = 16
NCH = 3
NCORES = 8
BATCH_PER_CORE = NBATCH // NCORES
IMGS = BATCH_PER_CORE * NCH  # 6 images per core
NCHUNK = H // 128  # 4

F32 = mybir.dt.float32
F16 = mybir.dt.float16

MAX_WAITS_PER_INST = 1


def _split_multi_waits(nc):
    """Rewrite instructions with >1 sem waits for this toolchain's walrus.

    The walrus codegen here rejects any instruction with more than one
    sync wait ("Too many sync wait commands", CoreV3GenImpl
    setupSyncWait). Surplus waits are moved onto freshly created nop
    instructions on the same engine, inserted immediately before the
    overloaded instruction — engine streams execute in order, so the
    guard is equivalent.
    """
    cur_bb = nc.cur_bb.bb
    for bb in nc.m.functions[0].blocks:
        out = []
        for inst in list(bb.instructions):
            si = inst.sync_info
            waits = list(si.on_wait) if si is not None and si.on_wait else []
            if len(waits) > MAX_WAITS_PER_INST:
                surplus = waits[:-MAX_WAITS_PER_INST]
                keep = waits[-MAX_WAITS_PER_INST:]
                upd = list(si.on_update) if si.on_update else []
                inst.sync_info = mybir.SyncInfo(on_wait=keep, on_update=upd)
                for w in surplus:
                    ni = nc.engines[inst.engine].nop().ins
                    assert cur_bb.instructions[-1] is ni
                    cur_bb.instructions.pop()
                    ni.sync_info = mybir.SyncInfo(on_wait=[w], on_update=[])
                    out.append(ni)
            out.append(inst)
        bb.instructions[:] = out


def _factor_kernel(w2d):
    """Rank-1 factor a (21,21) kernel: w2d[i,j] = wr[i]*wc[j]."""
    u, s, vt = np.linalg.svd(w2d.astype(np.float64))
    wr = u[:, 0] * np.sqrt(s[0])
    wc = vt[0] * np.sqrt(s[0])
    if wr.sum() < 0:
        wr, wc = -wr, -wc
    resid = np.abs(np.outer(wr, wc) - w2d).max()
    scale = max(np.abs(w2d).max(), 1e-30)
    assert resid <= 1e-4 * scale, f"kernel not separable: resid={resid}, scale={scale}"
    return wr, wc


def _band(w1d):
    """(21,) taps -> (512,512) f32 band matrix with reflection folded.

    B[r, n] accumulates every tap of output position n whose reflected
    source row is r:  out[n] = sum_r B[r, n] * x[r].
    """
    b = np.zeros((H, H), np.float64)
    for k in range(KSIZE):
        n = np.arange(H)
        r = n + k - PAD
        r = np.where(r < 0, -r, r)
        r = np.where(r >= H, 2 * H - 2 - r, r)
        np.add.at(b, (r, n), w1d[k])
    return np.ascontiguousarray(b.astype(np.float32))


def _pack_band(b):
    """Pack the nonzero output-column range of each 128-row chunk.

    Returns (packed [128, total_w] fp16, ranges [(n0,n1)], offsets).
    """
    ranges, offs, cols = [], [], []
    off = 0
    for j in range(NCHUNK):
        chunk = b[128 * j : 128 * (j + 1)]
        nz = np.flatnonzero(np.abs(chunk).max(axis=0) > 0)
        n0, n1 = int(nz[0]), int(nz[-1]) + 1
        ranges.append((n0, n1))
        offs.append(off)
        cols.append(chunk[:, n0:n1])
        off += n1 - n0
    packed = np.ascontiguousarray(np.concatenate(cols, axis=1).astype(np.float16))
    return packed, ranges, offs


def _build_program(ranges, offs, total_w):
    nc = bass.Bass("TRN2", target_bir_lowering=False, debug=False)
    x = nc.dram_tensor("x", [IMGS, 128, NCHUNK, H], F16, kind="ExternalInput").ap()
    bp = nc.dram_tensor("bp", [128, total_w], F16, kind="ExternalInput").ap()
    y = nc.dram_tensor("y", [IMGS, 128, NCHUNK, H], F16, kind="ExternalOutput").ap()

    with tile.TileContext(nc) as tc:
        with (
            tc.tile_pool(name="band", bufs=1) as band_pool,
            tc.tile_pool(name="warm", bufs=1) as warm_pool,
            tc.tile_pool(name="xin", bufs=IMGS) as xpool,
            tc.tile_pool(name="t1", bufs=3) as t1pool,
            tc.tile_pool(name="yout", bufs=IMGS) as ypool,
            tc.tile_pool(name="p1", bufs=2, space="PSUM") as p1pool,
            tc.tile_pool(name="p2", bufs=2, space="PSUM") as p2pool,
        ):
            b_s = band_pool.tile([128, total_w], F16, tag="bp")
            scratch = warm_pool.tile([128, 192], F16, tag="scratch")
            # Tiles must be written before read; DVE starts earliest and
            # a small memset keeps the warm matmuls' launch time low.
            nc.vector.memset(scratch[:, :], 0.0)
            xs = [
                xpool.tile([128, NCHUNK, H], F16, tag="xs", name=f"xs{i}")
                for i in range(IMGS)
            ]

            # All input DMAs up-front on SP (the fastest DMA issuer;
            # ACT/DVE hold their sequencer ~1.3us per DMA). Image 0 is
            # split in half so its first groups wait on just half of it.
            nc.gpsimd.dma_start(b_s[:, :], bp[:, :])
            nc.sync.dma_start(xs[0][:, 0:2, :], x[0, :, 0:2, :])
            nc.sync.dma_start(xs[0][:, 2:4, :], x[0, :, 2:4, :])
            nc.sync.dma_start(xs[1][:, 0:2, :], x[1, :, 0:2, :])
            nc.sync.dma_start(xs[1][:, 2:4, :], x[1, :, 2:4, :])
            for i in range(2, IMGS):
                nc.sync.dma_start(xs[i][:, :, :], x[i, :, :, :])

            def warm(dpair, cols):
                """Dummy matmuls into a scratch PSUM pair: keep the PE
                busy through its 3us p-state ramp / input-DMA waits. The
                scratch SBUF is never written (garbage is fine) and each
                target bank is later re-written with start=True.
                """
                for k, w in enumerate(cols):
                    nc.tensor.matmul(
                        dpair[:, k % 2, 0:w],
                        scratch[:, 0:128],
                        scratch[:, 0:w],
                        start=True,
                        stop=True,
                        skip_group_check=True,
                    )



            t1s, p2s_of, ys_of = {}, {}, {}

            # Image 0's PSUM pairs are allocated up-front so the warm
            # dummies can target them with no dependencies; the real
            # groups re-write every element with start=True semantics.
            p1s0 = [
                p1pool.tile([128, 2, H], F32, tag="p1", name=f"p1w_{h}")
                for h in range(2)
            ]
            p2s0 = [
                p2pool.tile([128, 2, H], F32, tag="p2", name=f"p2w_{h}")
                for h in range(2)
            ]

            def pass1(i, p1s=None):
                # t1 = x.T @ B: j-outer over 4 concurrent PSUM banks, so
                # PE work is available as soon as x chunk j lands.
                if p1s is None:
                    p1s = [
                        p1pool.tile([128, 2, H], F32, tag="p1", name=f"p1_{h}")
                        for h in range(2)
                    ]
                # Image 1 runs m-outer with the DVE-drained pair (m2,
                # m3) first: its t1B copy then starts ~500ns sooner,
                # shifting the whole DVE-packed block (and the drain)
                # left. Other images stream j-outer (image 0 needs it
                # for split-input overlap; later images are work-bound).
                if i in (1, 2):
                    loop = [(j, m) for m in (2, 3, 0, 1) for j in range(NCHUNK)]
                else:
                    loop = [(j, m) for j in range(NCHUNK) for m in range(NCHUNK)]
                for j, m in loop:
                    n0, n1 = ranges[j]
                    nc.tensor.matmul(
                        p1s[m // 2][:, m % 2, n0:n1],
                        xs[i][:, j, 128 * m : 128 * (m + 1)],
                        b_s[:, offs[j] : offs[j] + (n1 - n0)],
                        start=(j == 0),
                        stop=(j == NCHUNK - 1),
                    )
                return p1s

            def t1_copies(i, p1s):
                t1 = t1pool.tile([128, NCHUNK, H], F16, tag="t1")
                t1s[i] = t1
                nc.scalar.copy(t1[:, 0:2, :], p1s[0][:, :, :])
                nc.vector.tensor_copy(t1[:, 2:4, :], p1s[1][:, :, :])

            def pass2(i):
                if i == 0:
                    p2s = p2s0
                else:
                    # The last image borrows the p1 banks: they are free
                    # for good once t1(last) drains, while the p2 banks
                    # would still be waiting on image 4's y copies.
                    pool = p1pool if i == IMGS - 1 else p2pool
                    p2s = [
                        pool.tile([128, 2, H], F32, tag="p1" if i == IMGS - 1 else "p2",
                                  name=f"p2_{h}")
                        for h in range(2)
                    ]
                p2s_of[i] = p2s
                t1 = t1s.pop(i)
                # Last image runs r-outer so its pair-A groups stop at
                # the halfway mark and the final y copies start sooner
                # (c-outer stops everything together at the very end).
                if i >= IMGS - 2:
                    loop = [(c, r) for r in range(NCHUNK) for c in range(NCHUNK)]
                else:
                    loop = [(c, r) for c in range(NCHUNK) for r in range(NCHUNK)]
                for c, r in loop:
                    n0, n1 = ranges[c]
                    nc.tensor.matmul(
                        p2s[r // 2][:, r % 2, n0:n1],
                        t1[:, c, 128 * r : 128 * (r + 1)],
                        b_s[:, offs[c] : offs[c] + (n1 - n0)],
                        start=(c == 0),
                        stop=(c == NCHUNK - 1),
                    )

            def y_copies(i):
                p2s = p2s_of.pop(i)
                ysb = ypool.tile([128, NCHUNK, H], F16, tag="ys")
                ys_of[i] = ysb
                # Direct fp32->fp16 pair drains (u64/Pool detours fail
                # the ISA check / PSUM access rules on this toolchain).
                nc.scalar.copy(ysb[:, 0:2, :], p2s[0][:, :, :])
                nc.vector.tensor_copy(ysb[:, 2:4, :], p2s[1][:, :, :])

            def out_dma(i):
                # Per-pair (last image: per-bank) DMAs, each with a
                # single sem wait, so output drains as soon as possible.
                ysb = ys_of.pop(i)
                nc.sync.dma_start(y[i, :, 0:2, :], ysb[:, 0:2, :])
                nc.sync.dma_start(y[i, :, 2:4, :], ysb[:, 2:4, :])

            # Software pipeline: pass2(i-1) between pass1(i) and its t1
            # copies; y(i-1) copies are queued on ACT/DVE ahead of the
            # t1(i) copies so finished images drain first.
            # A few early warm dummies start the PE's p-state ramp
            # clock (~1.5us, right after the DVE memset): the cost model
            # reaches full speed 3us after the first busy period, and
            # idle gaps do not reset it.
            warm(p1s0[0], [96] * 2)
            warm(p1s0[1], [96] * 2)

            p1s_cur = None
            for i in range(IMGS):
                p1s_cur = pass1(i, p1s0 if i == 0 else None)
                if i > 0:
                    pass2(i - 1)
                t1_copies(i, p1s_cur)
                if i > 0:
                    y_copies(i - 1)
                    out_dma(i - 1)
            pass2(IMGS - 1)
            y_copies(IMGS - 1)
            out_dma(IMGS - 1)

    _split_multi_waits(nc)
    return nc


def _prepare(x, W):
    assert x.shape == (NBATCH, NCH, H, H), x.shape
    assert W.shape == (NCH, 1, KSIZE, KSIZE), W.shape
    w0 = np.asarray(W[0, 0], np.float32)
    for c in range(1, NCH):
        assert np.array_equal(np.asarray(W[c, 0], np.float32), w0), (
            "per-channel kernels differ; single-band path only"
        )
    wr, wc = _factor_kernel(w0)
    bv = _band(wr)
    bh = _band(wc)
    assert np.array_equal(bv, bh), "asymmetric kernel; shared-band path only"
    return bv


def _run(x, W, **spmd_kwargs):
    x = np.asarray(x, np.float32)
    bv = _prepare(x, W)
    packed, ranges, offs = _pack_band(bv)
    nc = _build_program(ranges, offs, packed.shape[1])

    # fp16 + permute rows so each image is one contiguous-per-partition
    # DMA: xd[i, p, j, c] = img[128j + p, c].
    x16 = x.astype(np.float16).reshape(NBATCH * NCH, NCHUNK, 128, H)
    in_maps = []
    for c in range(NCORES):
        shard = x16[c * IMGS : (c + 1) * IMGS].transpose(0, 2, 1, 3)
        in_maps.append({"x": np.ascontiguousarray(shard), "bp": packed})

    res = run_bass_kernel_spmd(nc, in_maps, list(range(NCORES)), **spmd_kwargs)
    out = np.empty((NBATCH * NCH, H, H), np.float32)
    for c in range(NCORES):
        yc = res.results[c]["y"]  # [IMGS, 128, NCHUNK, H] fp16
        out[c * IMGS : (c + 1) * IMGS] = (
            yc.transpose(0, 2, 1, 3).reshape(IMGS, H, H).astype(np.float32)
        )
    return out.reshape(NBATCH, NCH, H, H), res


def kernel(x, W):
    return _run(x, W)[0]
